# revision 8
# baseline (speedup 1.0000x reference)
"""Coverage-attention kernel for 8 TRN2 NeuronCores.

Data-parallel over batch: core i handles batches [i*BL, (i+1)*BL).
Weights replicated. No collectives needed.

Math per batch b (S source positions, H=512, 2H=1024):
    att^T[h, s] = tanh( sum_k W_h[k, h] * X[s, k] + W_c[h] * cov[s] + dec_feat[h] )
    e[s]        = sum_h v[h] * att^T[h, s] + masklog[s]      (masklog = (mask-1)*1e4)
    w[s]        = exp(e[s])        (masked entries underflow to exactly 0)
    Z           = sum_s w[s]
    ctx[k]      = sum_s X[s, k] * w[s] / Z
    att_dist[s] = w[s] / Z
    new_cov[s]  = cov[s] + att_dist[s]

X is uploaded host-pre-transposed (k on partitions) and pre-cast to bf16 so the
big matmul streams it directly at full TensorE rate; the context reduction over
s runs on the Vector engine (scalar_tensor_tensor with accum_out along the free
axis) over the same SBUF-resident tiles, so X is read from HBM exactly once.
All softmax/normalization arithmetic stays fp32.
"""

import numpy as np

N_CORES = 8
B, S, H = 64, 2048, 512
K2H = 2 * H
BL = B // N_CORES          # batches per core
NKC = K2H // 128           # 8 k-chunks
NHC = H // 128             # 4 h-chunks
ST = 512                   # s-tile width
NST = S // ST              # s-tiles per batch
XT_W = 512                 # xt SBUF tile width (1 s-tile)


def _build_nc(bl, s_len):
    import concourse.bacc as bacc
    import concourse.tile as tile
    from concourse import mybir

    FP = mybir.dt.float32
    BF = mybir.dt.bfloat16
    ALU = mybir.AluOpType
    ACT = mybir.ActivationFunctionType
    AX = mybir.AxisListType

    nst = s_len // ST
    nxt = s_len // XT_W

    nc = bacc.Bacc(None, debug=False)

    xt_ext = nc.declare_dram_parameter("xt", [bl, K2H, s_len], BF, isOutput=False)
    mlog_ext = nc.declare_dram_parameter("masklog", [bl, s_len], BF, isOutput=False)
    covb_ext = nc.declare_dram_parameter("covb", [bl, s_len], BF, isOutput=False)
    cov_ext = nc.declare_dram_parameter("cov", [bl, s_len], FP, isOutput=False)
    dst_ext = nc.declare_dram_parameter("dst", [128, 4 * bl], BF, isOutput=False)
    wh_ext = nc.declare_dram_parameter("wh", [128, NKC * H], BF, isOutput=False)
    ws_ext = nc.declare_dram_parameter("ws", [128, 4 * H], BF, isOutput=False)
    bs_ext = nc.declare_dram_parameter("bs", [1, H], BF, isOutput=False)
    wcc_ext = nc.declare_dram_parameter("wcc", [128, NHC], FP, isOutput=False)
    v_ext = nc.declare_dram_parameter("v", [1, H], BF, isOutput=False)
    ones_ext = nc.declare_dram_parameter("ones", [1, 128], BF, isOutput=False)

    ctx_ext = nc.declare_dram_parameter("ctx", [bl, K2H], FP, isOutput=True)
    att_ext = nc.declare_dram_parameter("att", [bl, s_len], FP, isOutput=True)
    ncov_ext = nc.declare_dram_parameter("ncov", [bl, s_len], FP, isOutput=True)

    with tile.TileContext(nc) as tc:
        with (
            tc.tile_pool(name="singles", bufs=1) as singles,
            tc.tile_pool(name="xt", bufs=2) as xtp,
            tc.tile_pool(name="work", bufs=2) as work,
            tc.tile_pool(name="perb", bufs=2) as perb,
            tc.tile_pool(name="ps_att", bufs=3, space="PSUM") as ps_att,
            tc.tile_pool(name="ps_e", bufs=2, space="PSUM") as ps_e,
            tc.tile_pool(name="ps_cov", bufs=2, space="PSUM") as ps_cov,
        ):
            wh_sbs = []
            for kc in range(NKC):
                whk = singles.tile([128, H], BF, tag=f"wh{kc}")
                nc.sync.dma_start(out=whk[:], in_=wh_ext[:, kc * H:(kc + 1) * H])
                wh_sbs.append(whk)
            ws_sb = singles.tile([128, 4 * H], BF)
            nc.sync.dma_start(out=ws_sb[:], in_=ws_ext[:])
            dst_sb = singles.tile([128, 4 * bl], BF)
            nc.sync.dma_start(out=dst_sb[:], in_=dst_ext[:])
            bs_sb = singles.tile([1, H], BF)
            nc.sync.dma_start(out=bs_sb[:], in_=bs_ext[:])
            wcc_sb = singles.tile([128, NHC], FP)
            nc.sync.dma_start(out=wcc_sb[:], in_=wcc_ext[:])
            v_sb = singles.tile([1, H], BF)
            nc.sync.dma_start(out=v_sb[:], in_=v_ext[:])
            ones_sb = singles.tile([1, 128], BF)
            nc.sync.dma_start(out=ones_sb[:], in_=ones_ext[:])

            # dec_feat^T[h, b] = sum_k W_s[k, h] * dec_state[b, k] + b_s[h]
            psd = ps_att.tile([128, 4 * bl], FP, tag="ps_att")
            for hc in range(NHC):
                o = hc * bl
                for kc2 in range(4):
                    nc.tensor.matmul(
                        psd[:, o:o + bl],
                        lhsT=ws_sb[:, kc2 * H + hc * 128: kc2 * H + hc * 128 + 128],
                        rhs=dst_sb[:, kc2 * bl:(kc2 + 1) * bl],
                        start=(kc2 == 0), stop=False,
                    )
                nc.tensor.matmul(
                    psd[:, o:o + bl],
                    lhsT=bs_sb[0:1, hc * 128:(hc + 1) * 128],
                    rhs=ones_sb[0:1, 0:bl],
                    start=False, stop=True,
                )
            dec_sb = singles.tile([128, 4 * bl], FP)
            nc.scalar.activation(dec_sb[:], psd[:], ACT.Copy)

            # v replicated along free dim: vrep[h_part, m] = v[hc*128 + h_part]
            psv = ps_att.tile([128, H], FP, tag="ps_att")
            for hc in range(NHC):
                nc.tensor.matmul(
                    psv[:, hc * 128:(hc + 1) * 128],
                    lhsT=v_sb[0:1, hc * 128:(hc + 1) * 128],
                    rhs=ones_sb[0:1, :],
                    start=True, stop=True,
                )
            vrep_sb = singles.tile([128, H], BF)
            nc.scalar.activation(vrep_sb[:], psv[:], ACT.Copy)

            for b in range(bl):
                mlog_row = perb.tile([1, s_len], BF, tag="mrow")
                nc.sync.dma_start(out=mlog_row[:], in_=mlog_ext[b:b + 1, :])
                covb_row = perb.tile([1, s_len], BF, tag="cbrow")
                nc.sync.dma_start(out=covb_row[:], in_=covb_ext[b:b + 1, :])
                cov_row = perb.tile([1, s_len], FP, tag="crow")
                nc.sync.dma_start(out=cov_row[:], in_=cov_ext[b:b + 1, :])
                acc_b = perb.tile([128, NKC * nst], FP, tag="acc")
                zcols = perb.tile([128, nst], FP, tag="zc")
                wrows = perb.tile([1, s_len], FP, tag="wrows")

                for g in range(nxt):
                    xts = []
                    for kc in range(NKC):
                        t = xtp.tile([128, XT_W], BF, tag=f"xt{kc}")
                        nc.sync.dma_start(
                            out=t[:],
                            in_=xt_ext[b, kc * 128:(kc + 1) * 128,
                                       g * XT_W:(g + 1) * XT_W],
                        )
                        xts.append(t)
                    for sst in range(XT_W // ST):
                        st = g * (XT_W // ST) + sst
                        ssl = slice(sst * ST, (sst + 1) * ST)
                        cvr = ps_cov.tile([128, ST], FP, tag="ps_cov")
                        nc.tensor.matmul(
                            cvr[:],
                            lhsT=ones_sb[0:1, :],
                            rhs=covb_row[0:1, st * ST:(st + 1) * ST],
                            start=True, stop=True,
                        )
                        cvs = work.tile([128, ST], FP, tag="cvs")
                        nc.scalar.activation(cvs[:], cvr[:], ACT.Copy)
                        attTs = []
                        for hc in range(NHC):
                            pa = ps_att.tile([128, ST], FP, tag="ps_att")
                            for kc in range(NKC):
                                nc.tensor.matmul(
                                    pa[:],
                                    lhsT=wh_sbs[kc][:, hc * 128:
                                                    hc * 128 + 128],
                                    rhs=xts[kc][:, ssl],
                                    start=(kc == 0), stop=(kc == NKC - 1),
                                )
                            nc.vector.scalar_tensor_tensor(
                                out=pa[:],
                                in0=cvs[:],
                                scalar=wcc_sb[:, hc:hc + 1],
                                in1=pa[:],
                                op0=ALU.mult,
                                op1=ALU.add,
                            )
                            at = work.tile([128, ST], BF, tag=f"attT{hc}")
                            nc.scalar.activation(
                                at[:], pa[:], ACT.Tanh,
                                bias=dec_sb[:, hc * bl + b: hc * bl + b + 1],
                            )
                            attTs.append(at)
                        pe_ = ps_e.tile([128, ST], FP, tag="ps_e")
                        for hc in range(NHC):
                            nc.tensor.matmul(
                                pe_[:],
                                lhsT=vrep_sb[:, hc * 128:(hc + 1) * 128],
                                rhs=attTs[hc][:],
                                start=(hc == 0), stop=False,
                            )
                        nc.tensor.matmul(
                            pe_[:],
                            lhsT=ones_sb[0:1, :],
                            rhs=mlog_row[0:1, st * ST:(st + 1) * ST],
                            start=False, stop=True,
                        )
                        wrep = work.tile([128, ST], BF, tag="wrep")
                        nc.scalar.activation(
                            wrep[:], pe_[:], ACT.Exp,
                            accum_out=zcols[:, st:st + 1],
                        )
                        nc.scalar.activation(
                            wrows[0:1, st * ST:(st + 1) * ST], wrep[0:1, :], ACT.Copy
                        )
                        trash = work.tile([128, ST], BF, tag="trash")
                        for kc in range(NKC):
                            nc.vector.scalar_tensor_tensor(
                                out=trash[:],
                                in0=xts[kc][:, ssl],
                                scalar=1.0,
                                in1=wrep[:],
                                op0=ALU.mult,
                                op1=ALU.mult,
                                accum_out=acc_b[:, kc * nst + st: kc * nst + st + 1],
                            )

                zred = perb.tile([128, 1], FP, tag="zred")
                nc.vector.tensor_reduce(zred[:], zcols[:], axis=AX.X, op=ALU.add)
                rz = perb.tile([128, 1], FP, tag="rz")
                nc.vector.reciprocal(rz[:], zred[:])
                ctxsum = perb.tile([128, NKC], FP, tag="ctxsum")
                nc.vector.tensor_reduce(
                    ctxsum[:],
                    acc_b[:].rearrange("p (kc st) -> p kc st", st=nst),
                    axis=AX.X, op=ALU.add,
                )
                ctxn = perb.tile([128, NKC], FP, tag="ctxn")
                nc.vector.tensor_scalar(
                    ctxn[:], ctxsum[:], rz[:, 0:1], None, op0=ALU.mult
                )
                nc.sync.dma_start(
                    out=ctx_ext[b].rearrange("(kc p) -> p kc", p=128),
                    in_=ctxn[:],
                )
                att_row = perb.tile([1, s_len], FP, tag="attrow")
                nc.scalar.activation(
                    att_row[:], wrows[:], ACT.Copy, scale=rz[0:1, 0:1]
                )
                nc.sync.dma_start(out=att_ext[b:b + 1, :], in_=att_row[:])
                ncov_row = perb.tile([1, s_len], FP, tag="ncovrow")
                nc.vector.scalar_tensor_tensor(
                    out=ncov_row[:],
                    in0=wrows[:],
                    scalar=rz[0:1, 0:1],
                    in1=cov_row[:],
                    op0=ALU.mult,
                    op1=ALU.add,
                )
                nc.sync.dma_start(out=ncov_ext[b:b + 1, :], in_=ncov_row[:])

    nc.compile()
    return nc


_NC_CACHE = {}


def _get_nc(bl, s_len):
    key = (bl, s_len)
    if key not in _NC_CACHE:
        _NC_CACHE[key] = _build_nc(bl, s_len)
    return _NC_CACHE[key]


def _prep_weights(W_h, W_s, b_s, W_c, v):
    import ml_dtypes
    BF = ml_dtypes.bfloat16
    return {
        "wh": np.ascontiguousarray(
            W_h.reshape(NKC, 128, H).transpose(1, 0, 2).reshape(128, NKC * H)
        ).astype(BF),
        "ws": np.ascontiguousarray(
            W_s.reshape(4, 128, H).transpose(1, 0, 2).reshape(128, 4 * H)
        ).astype(BF),
        "bs": b_s.reshape(1, H).astype(BF),
        "wcc": np.ascontiguousarray(W_c.reshape(NHC, 128).T, dtype=np.float32),
        "v": v.reshape(H)[None, :].astype(BF),
        "ones": np.ones((1, 128), dtype=BF),
    }


def _prep_core_inputs(weights, decoder_state, encoder_outputs, encoder_mask,
                      coverage, lo, hi):
    import ml_dtypes
    BF = ml_dtypes.bfloat16
    bl = hi - lo
    xt = np.ascontiguousarray(
        encoder_outputs[lo:hi].transpose(0, 2, 1)
    ).astype(BF)
    masklog = ((encoder_mask[lo:hi].astype(np.float32)) - 1.0) * 10000.0
    cov = np.ascontiguousarray(coverage[lo:hi], dtype=np.float32)
    dst = np.ascontiguousarray(
        decoder_state[lo:hi].T.reshape(4, 128, bl).transpose(1, 0, 2)
        .reshape(128, 4 * bl)
    ).astype(BF)
    d = {
        "xt": xt,
        "masklog": masklog.astype(BF),
        "covb": cov.astype(BF),
        "cov": cov,
        "dst": dst,
    }
    d.update(weights)
    return d


def run_on_cores(decoder_state, encoder_outputs, encoder_mask, coverage,
                 W_h, W_s, b_s, W_c, v, trace=False, tmpdir=None):
    from concourse.bass_utils import run_bass_kernel_spmd

    b_total = encoder_outputs.shape[0]
    s_len = encoder_outputs.shape[1]
    bl = b_total // N_CORES
    nc = _get_nc(bl, s_len)
    weights = _prep_weights(W_h, W_s, b_s, W_c, v)
    in_maps = [
        _prep_core_inputs(weights, decoder_state, encoder_outputs,
                          encoder_mask, coverage, i * bl, (i + 1) * bl)
        for i in range(N_CORES)
    ]
    res = run_bass_kernel_spmd(
        nc, in_maps, core_ids=list(range(N_CORES)), trace=trace, tmpdir=tmpdir
    )
    ctx = np.concatenate([res.results[i]["ctx"] for i in range(N_CORES)], axis=0)
    att = np.concatenate([res.results[i]["att"] for i in range(N_CORES)], axis=0)
    ncov = np.concatenate([res.results[i]["ncov"] for i in range(N_CORES)], axis=0)
    return (ctx, att, ncov), res


def kernel(decoder_state, encoder_outputs, encoder_mask, coverage,
           W_h, W_s, b_s, W_c, v):
    outs, _ = run_on_cores(
        np.asarray(decoder_state), np.asarray(encoder_outputs),
        np.asarray(encoder_mask), np.asarray(coverage),
        np.asarray(W_h), np.asarray(W_s), np.asarray(b_s),
        np.asarray(W_c), np.asarray(v),
    )
    return outs


# revision 9
# speedup vs baseline: 1.1774x; 1.1774x over previous
"""Coverage-attention kernel for 8 TRN2 NeuronCores.

Data-parallel over batch: core i handles batches [i*BL, (i+1)*BL).
Weights replicated. No collectives needed.

Math per batch b (S source positions, H=512, 2H=1024):
    att^T[h, s] = tanh( sum_k W_h[k, h] * X[s, k] + W_c[h] * cov[s] + dec_feat[h] )
    e[s]        = sum_h v[h] * att^T[h, s] + masklog[s]      (masklog = (mask-1)*1e4)
    w[s]        = exp(e[s])        (masked entries underflow to exactly 0)
    Z           = sum_s w[s]
    ctx[k]      = sum_s X[s, k] * w[s] / Z
    att_dist[s] = w[s] / Z
    new_cov[s]  = cov[s] + att_dist[s]

X is uploaded host-pre-transposed (k on partitions) and pre-cast to bf16 so the
big matmul streams it directly at full TensorE rate; the context reduction over
s runs on the Vector engine (scalar_tensor_tensor with accum_out along the free
axis) over the same SBUF-resident tiles, so X is read from HBM exactly once.
All softmax/normalization arithmetic stays fp32.
"""

import numpy as np

N_CORES = 8
B, S, H = 64, 2048, 512
K2H = 2 * H
BL = B // N_CORES          # batches per core
NKC = K2H // 128           # 8 k-chunks
NHC = H // 128             # 4 h-chunks
ST = 512                   # s-tile width
NST = S // ST              # s-tiles per batch
XT_W = 1024                # xt SBUF tile width (2 s-tiles)


def _build_nc(bl, s_len):
    import concourse.bacc as bacc
    import concourse.tile as tile
    from concourse import mybir

    FP = mybir.dt.float32
    BF = mybir.dt.bfloat16
    ALU = mybir.AluOpType
    ACT = mybir.ActivationFunctionType
    AX = mybir.AxisListType

    nst = s_len // ST
    nxt = s_len // XT_W

    nc = bacc.Bacc(None, debug=False)

    xt_ext = nc.declare_dram_parameter("xt", [bl, K2H, s_len], BF, isOutput=False)
    mlog_ext = nc.declare_dram_parameter("masklog", [bl, s_len], BF, isOutput=False)
    covb_ext = nc.declare_dram_parameter("covb", [bl, s_len], BF, isOutput=False)
    cov_ext = nc.declare_dram_parameter("cov", [bl, s_len], FP, isOutput=False)
    dst_ext = nc.declare_dram_parameter("dst", [128, 4 * bl], BF, isOutput=False)
    wh_ext = nc.declare_dram_parameter("wh", [128, NKC * H], BF, isOutput=False)
    ws_ext = nc.declare_dram_parameter("ws", [128, 4 * H], BF, isOutput=False)
    bs_ext = nc.declare_dram_parameter("bs", [1, H], BF, isOutput=False)
    wcc_ext = nc.declare_dram_parameter("wcc", [128, NHC], FP, isOutput=False)
    v_ext = nc.declare_dram_parameter("v", [1, H], BF, isOutput=False)
    ones_ext = nc.declare_dram_parameter("ones", [1, 128], BF, isOutput=False)

    ctx_ext = nc.declare_dram_parameter("ctx", [bl, K2H], FP, isOutput=True)
    att_ext = nc.declare_dram_parameter("att", [bl, s_len], FP, isOutput=True)
    ncov_ext = nc.declare_dram_parameter("ncov", [bl, s_len], FP, isOutput=True)

    with tile.TileContext(nc) as tc:
        with (
            tc.tile_pool(name="singles", bufs=1) as singles,
            tc.tile_pool(name="xt", bufs=2) as xtp,
            tc.tile_pool(name="work", bufs=2) as work,
            tc.tile_pool(name="perb", bufs=2) as perb,
            tc.tile_pool(name="ps_att", bufs=3, space="PSUM") as ps_att,
            tc.tile_pool(name="ps_e", bufs=2, space="PSUM") as ps_e,
            tc.tile_pool(name="ps_cov", bufs=2, space="PSUM") as ps_cov,
        ):
            wh_sbs = []
            for kc in range(NKC):
                whk = singles.tile([128, H], BF, tag=f"wh{kc}")
                nc.sync.dma_start(out=whk[:], in_=wh_ext[:, kc * H:(kc + 1) * H])
                wh_sbs.append(whk)
            ws_sb = singles.tile([128, 4 * H], BF)
            nc.sync.dma_start(out=ws_sb[:], in_=ws_ext[:])
            dst_sb = singles.tile([128, 4 * bl], BF)
            nc.sync.dma_start(out=dst_sb[:], in_=dst_ext[:])
            bs_sb = singles.tile([1, H], BF)
            nc.sync.dma_start(out=bs_sb[:], in_=bs_ext[:])
            wcc_sb = singles.tile([128, NHC], FP)
            nc.sync.dma_start(out=wcc_sb[:], in_=wcc_ext[:])
            v_sb = singles.tile([1, H], BF)
            nc.sync.dma_start(out=v_sb[:], in_=v_ext[:])
            ones_sb = singles.tile([1, 128], BF)
            nc.sync.dma_start(out=ones_sb[:], in_=ones_ext[:])

            # dec_feat^T[h, b] = sum_k W_s[k, h] * dec_state[b, k] + b_s[h]
            psd = ps_att.tile([128, 4 * bl], FP, tag="ps_att")
            for hc in range(NHC):
                o = hc * bl
                for kc2 in range(4):
                    nc.tensor.matmul(
                        psd[:, o:o + bl],
                        lhsT=ws_sb[:, kc2 * H + hc * 128: kc2 * H + hc * 128 + 128],
                        rhs=dst_sb[:, kc2 * bl:(kc2 + 1) * bl],
                        start=(kc2 == 0), stop=False,
                    )
                nc.tensor.matmul(
                    psd[:, o:o + bl],
                    lhsT=bs_sb[0:1, hc * 128:(hc + 1) * 128],
                    rhs=ones_sb[0:1, 0:bl],
                    start=False, stop=True,
                )
            dec_sb = singles.tile([128, 4 * bl], FP)
            nc.scalar.activation(dec_sb[:], psd[:], ACT.Copy)

            # v replicated along free dim: vrep[h_part, m] = v[hc*128 + h_part]
            psv = ps_att.tile([128, H], FP, tag="ps_att")
            for hc in range(NHC):
                nc.tensor.matmul(
                    psv[:, hc * 128:(hc + 1) * 128],
                    lhsT=v_sb[0:1, hc * 128:(hc + 1) * 128],
                    rhs=ones_sb[0:1, :],
                    start=True, stop=True,
                )
            vrep_sb = singles.tile([128, H], BF)
            nc.scalar.activation(vrep_sb[:], psv[:], ACT.Copy)

            for b in range(bl):
                mlog_row = perb.tile([1, s_len], BF, tag="mrow")
                nc.sync.dma_start(out=mlog_row[:], in_=mlog_ext[b:b + 1, :])
                covb_row = perb.tile([1, s_len], BF, tag="cbrow")
                nc.sync.dma_start(out=covb_row[:], in_=covb_ext[b:b + 1, :])
                cov_row = perb.tile([1, s_len], FP, tag="crow")
                nc.sync.dma_start(out=cov_row[:], in_=cov_ext[b:b + 1, :])
                acc_b = perb.tile([128, NKC * nst], FP, tag="acc")
                zcols = perb.tile([128, nst], FP, tag="zc")
                wrows = perb.tile([1, s_len], FP, tag="wrows")

                for g in range(nxt):
                    xts = []
                    for kc in range(NKC):
                        t = xtp.tile([128, XT_W], BF, tag=f"xt{kc}")
                        nc.sync.dma_start(
                            out=t[:],
                            in_=xt_ext[b, kc * 128:(kc + 1) * 128,
                                       g * XT_W:(g + 1) * XT_W],
                        )
                        xts.append(t)
                    for sst in range(XT_W // ST):
                        st = g * (XT_W // ST) + sst
                        ssl = slice(sst * ST, (sst + 1) * ST)
                        cvr = ps_cov.tile([128, ST], FP, tag="ps_cov")
                        nc.tensor.matmul(
                            cvr[:],
                            lhsT=ones_sb[0:1, :],
                            rhs=covb_row[0:1, st * ST:(st + 1) * ST],
                            start=True, stop=True,
                        )
                        cvs = work.tile([128, ST], FP, tag="cvs")
                        nc.scalar.activation(cvs[:], cvr[:], ACT.Copy)
                        attTs = []
                        for hc in range(NHC):
                            pa = ps_att.tile([128, ST], FP, tag="ps_att")
                            for kc in range(NKC):
                                nc.tensor.matmul(
                                    pa[:],
                                    lhsT=wh_sbs[kc][:, hc * 128:
                                                    hc * 128 + 128],
                                    rhs=xts[kc][:, ssl],
                                    start=(kc == 0), stop=(kc == NKC - 1),
                                )
                            nc.vector.scalar_tensor_tensor(
                                out=pa[:],
                                in0=cvs[:],
                                scalar=wcc_sb[:, hc:hc + 1],
                                in1=pa[:],
                                op0=ALU.mult,
                                op1=ALU.add,
                            )
                            at = work.tile([128, ST], BF, tag=f"attT{hc}")
                            nc.scalar.activation(
                                at[:], pa[:], ACT.Tanh,
                                bias=dec_sb[:, hc * bl + b: hc * bl + b + 1],
                            )
                            attTs.append(at)
                        pe_ = ps_e.tile([128, ST], FP, tag="ps_e")
                        for hc in range(NHC):
                            nc.tensor.matmul(
                                pe_[:],
                                lhsT=vrep_sb[:, hc * 128:(hc + 1) * 128],
                                rhs=attTs[hc][:],
                                start=(hc == 0), stop=False,
                            )
                        nc.tensor.matmul(
                            pe_[:],
                            lhsT=ones_sb[0:1, :],
                            rhs=mlog_row[0:1, st * ST:(st + 1) * ST],
                            start=False, stop=True,
                        )
                        wrep = work.tile([128, ST], FP, tag="wrep")
                        nc.scalar.activation(
                            wrep[:], pe_[:], ACT.Exp,
                            accum_out=zcols[:, st:st + 1],
                        )
                        nc.scalar.activation(
                            wrows[0:1, st * ST:(st + 1) * ST], wrep[0:1, :], ACT.Copy
                        )
                        trash = work.tile([128, ST], FP, tag="trash")
                        for kc in range(NKC):
                            nc.vector.scalar_tensor_tensor(
                                out=trash[:],
                                in0=xts[kc][:, ssl],
                                scalar=1.0,
                                in1=wrep[:],
                                op0=ALU.mult,
                                op1=ALU.mult,
                                accum_out=acc_b[:, kc * nst + st: kc * nst + st + 1],
                            )

                zred = perb.tile([128, 1], FP, tag="zred")
                nc.vector.tensor_reduce(zred[:], zcols[:], axis=AX.X, op=ALU.add)
                rz = perb.tile([128, 1], FP, tag="rz")
                nc.vector.reciprocal(rz[:], zred[:])
                ctxsum = perb.tile([128, NKC], FP, tag="ctxsum")
                nc.vector.tensor_reduce(
                    ctxsum[:],
                    acc_b[:].rearrange("p (kc st) -> p kc st", st=nst),
                    axis=AX.X, op=ALU.add,
                )
                ctxn = perb.tile([128, NKC], FP, tag="ctxn")
                nc.vector.tensor_scalar(
                    ctxn[:], ctxsum[:], rz[:, 0:1], None, op0=ALU.mult
                )
                nc.sync.dma_start(
                    out=ctx_ext[b].rearrange("(kc p) -> p kc", p=128),
                    in_=ctxn[:],
                )
                att_row = perb.tile([1, s_len], FP, tag="attrow")
                nc.scalar.activation(
                    att_row[:], wrows[:], ACT.Copy, scale=rz[0:1, 0:1]
                )
                nc.sync.dma_start(out=att_ext[b:b + 1, :], in_=att_row[:])
                ncov_row = perb.tile([1, s_len], FP, tag="ncovrow")
                nc.vector.scalar_tensor_tensor(
                    out=ncov_row[:],
                    in0=wrows[:],
                    scalar=rz[0:1, 0:1],
                    in1=cov_row[:],
                    op0=ALU.mult,
                    op1=ALU.add,
                )
                nc.sync.dma_start(out=ncov_ext[b:b + 1, :], in_=ncov_row[:])

    nc.compile()
    return nc


_NC_CACHE = {}


def _get_nc(bl, s_len):
    key = (bl, s_len)
    if key not in _NC_CACHE:
        _NC_CACHE[key] = _build_nc(bl, s_len)
    return _NC_CACHE[key]


def _prep_weights(W_h, W_s, b_s, W_c, v):
    import ml_dtypes
    BF = ml_dtypes.bfloat16
    return {
        "wh": np.ascontiguousarray(
            W_h.reshape(NKC, 128, H).transpose(1, 0, 2).reshape(128, NKC * H)
        ).astype(BF),
        "ws": np.ascontiguousarray(
            W_s.reshape(4, 128, H).transpose(1, 0, 2).reshape(128, 4 * H)
        ).astype(BF),
        "bs": b_s.reshape(1, H).astype(BF),
        "wcc": np.ascontiguousarray(W_c.reshape(NHC, 128).T, dtype=np.float32),
        "v": v.reshape(H)[None, :].astype(BF),
        "ones": np.ones((1, 128), dtype=BF),
    }


def _prep_core_inputs(weights, decoder_state, encoder_outputs, encoder_mask,
                      coverage, lo, hi):
    import ml_dtypes
    BF = ml_dtypes.bfloat16
    bl = hi - lo
    xt = np.ascontiguousarray(
        encoder_outputs[lo:hi].transpose(0, 2, 1)
    ).astype(BF)
    masklog = ((encoder_mask[lo:hi].astype(np.float32)) - 1.0) * 10000.0
    cov = np.ascontiguousarray(coverage[lo:hi], dtype=np.float32)
    dst = np.ascontiguousarray(
        decoder_state[lo:hi].T.reshape(4, 128, bl).transpose(1, 0, 2)
        .reshape(128, 4 * bl)
    ).astype(BF)
    d = {
        "xt": xt,
        "masklog": masklog.astype(BF),
        "covb": cov.astype(BF),
        "cov": cov,
        "dst": dst,
    }
    d.update(weights)
    return d


def run_on_cores(decoder_state, encoder_outputs, encoder_mask, coverage,
                 W_h, W_s, b_s, W_c, v, trace=False, tmpdir=None):
    from concourse.bass_utils import run_bass_kernel_spmd

    b_total = encoder_outputs.shape[0]
    s_len = encoder_outputs.shape[1]
    bl = b_total // N_CORES
    nc = _get_nc(bl, s_len)
    weights = _prep_weights(W_h, W_s, b_s, W_c, v)
    in_maps = [
        _prep_core_inputs(weights, decoder_state, encoder_outputs,
                          encoder_mask, coverage, i * bl, (i + 1) * bl)
        for i in range(N_CORES)
    ]
    res = run_bass_kernel_spmd(
        nc, in_maps, core_ids=list(range(N_CORES)), trace=trace, tmpdir=tmpdir
    )
    ctx = np.concatenate([res.results[i]["ctx"] for i in range(N_CORES)], axis=0)
    att = np.concatenate([res.results[i]["att"] for i in range(N_CORES)], axis=0)
    ncov = np.concatenate([res.results[i]["ncov"] for i in range(N_CORES)], axis=0)
    return (ctx, att, ncov), res


def kernel(decoder_state, encoder_outputs, encoder_mask, coverage,
           W_h, W_s, b_s, W_c, v):
    outs, _ = run_on_cores(
        np.asarray(decoder_state), np.asarray(encoder_outputs),
        np.asarray(encoder_mask), np.asarray(coverage),
        np.asarray(W_h), np.asarray(W_s), np.asarray(b_s),
        np.asarray(W_c), np.asarray(v),
    )
    return outs


# revision 15
# speedup vs baseline: 1.3926x; 1.1828x over previous
"""Coverage-attention kernel for 8 TRN2 NeuronCores.

Data-parallel over batch: core i handles batches [i*BL, (i+1)*BL).
Weights replicated. No collectives needed.

Math per batch b (S source positions, H=512, 2H=1024):
    att^T[h, s] = tanh( sum_k W_h[k, h] * X[s, k] + W_c[h] * cov[s] + dec_feat[h] )
    e[s]        = sum_h v[h] * att^T[h, s] + masklog[s]      (masklog = (mask-1)*1e4)
    w[s]        = exp(e[s])        (masked entries underflow to exactly 0)
    Z           = sum_s w[s]
    ctx[k]      = sum_s X[s, k] * w[s] / Z
    att_dist[s] = w[s] / Z
    new_cov[s]  = cov[s] + att_dist[s]

X is uploaded host-pre-transposed (k on partitions) and pre-cast to bf16 so the
big matmul streams it directly at full TensorE rate; the context reduction over
s runs on the Vector engine (scalar_tensor_tensor with accum_out along the free
axis) over the same SBUF-resident tiles, so X is read from HBM exactly once.
All softmax/normalization arithmetic stays fp32.
"""

import numpy as np

N_CORES = 8
B, S, H = 64, 2048, 512
K2H = 2 * H
BL = B // N_CORES          # batches per core
NKC = K2H // 128           # 8 k-chunks
NHC = H // 128             # 4 h-chunks
ST = 512                   # s-tile width
NST = S // ST              # s-tiles per batch
XT_W = 2048                # xt SBUF tile width (4 s-tiles)


def _build_nc(bl, s_len):
    import concourse.bacc as bacc
    import concourse.tile as tile
    from concourse import mybir

    FP = mybir.dt.float32
    BF = mybir.dt.bfloat16
    ALU = mybir.AluOpType
    ACT = mybir.ActivationFunctionType
    AX = mybir.AxisListType

    nst = s_len // ST
    nxt = s_len // XT_W

    nc = bacc.Bacc(None, debug=False)

    xt_ext = nc.declare_dram_parameter("xt", [bl, K2H, s_len], BF, isOutput=False)
    mlog_ext = nc.declare_dram_parameter("masklog", [bl, s_len], BF, isOutput=False)
    covb_ext = nc.declare_dram_parameter("covb", [bl, s_len], BF, isOutput=False)
    cov_ext = nc.declare_dram_parameter("cov", [bl, s_len], FP, isOutput=False)
    dst_ext = nc.declare_dram_parameter("dst", [128, 4 * bl], BF, isOutput=False)
    wh_ext = nc.declare_dram_parameter("wh", [128, NKC * H], BF, isOutput=False)
    ws_ext = nc.declare_dram_parameter("ws", [128, 4 * H], BF, isOutput=False)
    bs_ext = nc.declare_dram_parameter("bs", [1, H], BF, isOutput=False)
    wcc_ext = nc.declare_dram_parameter("wcc", [128, NHC], FP, isOutput=False)
    v_ext = nc.declare_dram_parameter("v", [1, H], BF, isOutput=False)
    ones_ext = nc.declare_dram_parameter("ones", [1, 128], BF, isOutput=False)

    ctx_ext = nc.declare_dram_parameter("ctx", [bl, K2H], FP, isOutput=True)
    att_ext = nc.declare_dram_parameter("att", [bl, s_len], FP, isOutput=True)
    ncov_ext = nc.declare_dram_parameter("ncov", [bl, s_len], FP, isOutput=True)

    with tile.TileContext(nc) as tc:
        with (
            tc.tile_pool(name="singles", bufs=1) as singles,
            tc.tile_pool(name="xt", bufs=2) as xtp,
            tc.tile_pool(name="work", bufs=2) as work,
            tc.tile_pool(name="perb", bufs=2) as perb,
            tc.tile_pool(name="ps_att", bufs=3, space="PSUM") as ps_att,
            tc.tile_pool(name="ps_e", bufs=2, space="PSUM") as ps_e,
            tc.tile_pool(name="ps_cov", bufs=2, space="PSUM") as ps_cov,
        ):
            wh_sbs = []
            for kc in range(NKC):
                whk = singles.tile([128, H], BF, tag=f"wh{kc}")
                nc.sync.dma_start(out=whk[:], in_=wh_ext[:, kc * H:(kc + 1) * H])
                wh_sbs.append(whk)
            ws_sb = singles.tile([128, 4 * H], BF)
            nc.sync.dma_start(out=ws_sb[:], in_=ws_ext[:])
            dst_sb = singles.tile([128, 4 * bl], BF)
            nc.sync.dma_start(out=dst_sb[:], in_=dst_ext[:])
            bs_sb = singles.tile([1, H], BF)
            nc.sync.dma_start(out=bs_sb[:], in_=bs_ext[:])
            wcc_sb = singles.tile([128, NHC], FP)
            nc.sync.dma_start(out=wcc_sb[:], in_=wcc_ext[:])
            v_sb = singles.tile([1, H], BF)
            nc.sync.dma_start(out=v_sb[:], in_=v_ext[:])
            ones_sb = singles.tile([1, 128], BF)
            nc.sync.dma_start(out=ones_sb[:], in_=ones_ext[:])

            # dec_feat^T[h, b] = sum_k W_s[k, h] * dec_state[b, k] + b_s[h]
            psd = ps_att.tile([128, 4 * bl], FP, tag="ps_att")
            for hc in range(NHC):
                o = hc * bl
                for kc2 in range(4):
                    nc.tensor.matmul(
                        psd[:, o:o + bl],
                        lhsT=ws_sb[:, kc2 * H + hc * 128: kc2 * H + hc * 128 + 128],
                        rhs=dst_sb[:, kc2 * bl:(kc2 + 1) * bl],
                        start=(kc2 == 0), stop=False,
                    )
                nc.tensor.matmul(
                    psd[:, o:o + bl],
                    lhsT=bs_sb[0:1, hc * 128:(hc + 1) * 128],
                    rhs=ones_sb[0:1, 0:bl],
                    start=False, stop=True,
                )
            dec_sb = singles.tile([128, 4 * bl], FP)
            nc.scalar.activation(dec_sb[:], psd[:], ACT.Copy)

            # v replicated along free dim: vrep[h_part, m] = v[hc*128 + h_part]
            psv = ps_att.tile([128, H], FP, tag="ps_att")
            for hc in range(NHC):
                nc.tensor.matmul(
                    psv[:, hc * 128:(hc + 1) * 128],
                    lhsT=v_sb[0:1, hc * 128:(hc + 1) * 128],
                    rhs=ones_sb[0:1, :],
                    start=True, stop=True,
                )
            vrep_sb = singles.tile([128, H], BF)
            nc.scalar.activation(vrep_sb[:], psv[:], ACT.Copy)

            for b in range(bl):
                mlog_row = perb.tile([1, s_len], BF, tag="mrow")
                nc.sync.dma_start(out=mlog_row[:], in_=mlog_ext[b:b + 1, :])
                covb_row = perb.tile([1, s_len], BF, tag="cbrow")
                nc.sync.dma_start(out=covb_row[:], in_=covb_ext[b:b + 1, :])
                cov4 = perb.tile([128, ST], FP, tag="crow")
                nc.sync.dma_start(
                    out=cov4[:].rearrange("(a c) s -> a c s", c=32)[:, 0:1, :],
                    in_=cov_ext[b].rearrange("(t o s) -> t o s", o=1, s=ST))
                acc_b = perb.tile([128, NKC * nst], FP, tag="acc")
                zcols = perb.tile([128, nst], FP, tag="zc")
                wrows = perb.tile([128, ST], FP, tag="wrows")

                for g in range(nxt):
                    xts = []
                    for kc in range(NKC):
                        t = xtp.tile([128, XT_W], BF, tag=f"xt{kc}")
                        if b == 0 and g == 0:
                            for q in range(XT_W // ST):
                                nc.sync.dma_start(
                                    out=t[:, q * ST:(q + 1) * ST],
                                    in_=xt_ext[b, kc * 128:(kc + 1) * 128,
                                               q * ST:(q + 1) * ST],
                                )
                        else:
                            nc.sync.dma_start(
                                out=t[:],
                                in_=xt_ext[b, kc * 128:(kc + 1) * 128,
                                           g * XT_W:(g + 1) * XT_W],
                            )
                        xts.append(t)
                    for sst in range(XT_W // ST):
                        st = g * (XT_W // ST) + sst
                        ssl = slice(sst * ST, (sst + 1) * ST)
                        cvr = ps_cov.tile([128, ST], FP, tag="ps_cov")
                        nc.tensor.matmul(
                            cvr[:],
                            lhsT=ones_sb[0:1, :],
                            rhs=covb_row[0:1, st * ST:(st + 1) * ST],
                            start=True, stop=True,
                        )
                        cvs = work.tile([128, ST], FP, tag="cvs")
                        nc.scalar.activation(cvs[:], cvr[:], ACT.Copy)
                        attTs = []
                        for hc in range(NHC):
                            pa = ps_att.tile([128, ST], FP, tag="ps_att")
                            for kc in range(NKC):
                                nc.tensor.matmul(
                                    pa[:],
                                    lhsT=wh_sbs[kc][:, hc * 128:
                                                    hc * 128 + 128],
                                    rhs=xts[kc][:, ssl],
                                    start=(kc == 0), stop=(kc == NKC - 1),
                                )
                            cva = work.tile([128, ST], FP, tag=f"cva{hc}")
                            nc.vector.scalar_tensor_tensor(
                                out=cva[:],
                                in0=cvs[:],
                                scalar=wcc_sb[:, hc:hc + 1],
                                in1=pa[:],
                                op0=ALU.mult,
                                op1=ALU.add,
                            )
                            at = work.tile([128, ST], BF, tag=f"attT{hc}")
                            nc.scalar.activation(
                                at[:], cva[:], ACT.Tanh,
                                bias=dec_sb[:, hc * bl + b: hc * bl + b + 1],
                            )
                            attTs.append(at)
                        pe_ = ps_e.tile([128, ST], FP, tag="ps_e")
                        for hc in range(NHC):
                            nc.tensor.matmul(
                                pe_[:],
                                lhsT=vrep_sb[:, hc * 128:(hc + 1) * 128],
                                rhs=attTs[hc][:],
                                start=(hc == 0), stop=False,
                            )
                        nc.tensor.matmul(
                            pe_[:],
                            lhsT=ones_sb[0:1, :],
                            rhs=mlog_row[0:1, st * ST:(st + 1) * ST],
                            start=False, stop=True,
                        )
                        wrep = work.tile([128, ST], FP, tag="wrep")
                        nc.scalar.activation(
                            wrep[:], pe_[:], ACT.Exp,
                            accum_out=zcols[:, st:st + 1],
                        )
                        nc.scalar.activation(
                            wrows[32 * st:32 * st + 1, :],
                            wrep[32 * st:32 * st + 1, :], ACT.Copy
                        )
                        trash = work.tile([128, ST], FP, tag="trash")
                        for kc in range(NKC):
                            nc.vector.scalar_tensor_tensor(
                                out=trash[:],
                                in0=xts[kc][:, ssl],
                                scalar=1.0,
                                in1=wrep[:],
                                op0=ALU.mult,
                                op1=ALU.mult,
                                accum_out=acc_b[:, kc * nst + st: kc * nst + st + 1],
                            )

                zred = perb.tile([128, 1], FP, tag="zred")
                nc.vector.tensor_reduce(zred[:], zcols[:], axis=AX.X, op=ALU.add)
                rz = perb.tile([128, 1], FP, tag="rz")
                nc.vector.reciprocal(rz[:], zred[:])
                ctxsum = perb.tile([128, NKC], FP, tag="ctxsum")
                nc.vector.tensor_reduce(
                    ctxsum[:],
                    acc_b[:].rearrange("p (kc st) -> p kc st", st=nst),
                    axis=AX.X, op=ALU.add,
                )
                ctxn = perb.tile([128, NKC], FP, tag="ctxn")
                nc.vector.tensor_scalar(
                    ctxn[:], ctxsum[:], rz[:, 0:1], None, op0=ALU.mult
                )
                nc.sync.dma_start(
                    out=ctx_ext[b].rearrange("(kc p) -> p kc", p=128),
                    in_=ctxn[:],
                )
                att_row = perb.tile([128, ST], FP, tag="attrow")
                nc.scalar.activation(
                    att_row[:], wrows[:], ACT.Copy, scale=rz[:, 0:1]
                )
                nc.sync.dma_start(
                    out=att_ext[b].rearrange("(t s) -> t s", s=ST),
                    in_=att_row[:].rearrange("(a c) s -> a c s", c=32)[:, 0, :])
                ncov_row = perb.tile([128, ST], FP, tag="ncovrow")
                nc.vector.scalar_tensor_tensor(
                    out=ncov_row[:],
                    in0=wrows[:],
                    scalar=rz[:, 0:1],
                    in1=cov4[:],
                    op0=ALU.mult,
                    op1=ALU.add,
                )
                nc.sync.dma_start(
                    out=ncov_ext[b].rearrange("(t s) -> t s", s=ST),
                    in_=ncov_row[:].rearrange("(a c) s -> a c s", c=32)[:, 0, :])

    nc.compile()
    return nc


_NC_CACHE = {}


def _get_nc(bl, s_len):
    key = (bl, s_len)
    if key not in _NC_CACHE:
        _NC_CACHE[key] = _build_nc(bl, s_len)
    return _NC_CACHE[key]


def _prep_weights(W_h, W_s, b_s, W_c, v):
    import ml_dtypes
    BF = ml_dtypes.bfloat16
    return {
        "wh": np.ascontiguousarray(
            W_h.reshape(NKC, 128, H).transpose(1, 0, 2).reshape(128, NKC * H)
        ).astype(BF),
        "ws": np.ascontiguousarray(
            W_s.reshape(4, 128, H).transpose(1, 0, 2).reshape(128, 4 * H)
        ).astype(BF),
        "bs": b_s.reshape(1, H).astype(BF),
        "wcc": np.ascontiguousarray(W_c.reshape(NHC, 128).T, dtype=np.float32),
        "v": v.reshape(H)[None, :].astype(BF),
        "ones": np.ones((1, 128), dtype=BF),
    }


def _prep_core_inputs(weights, decoder_state, encoder_outputs, encoder_mask,
                      coverage, lo, hi):
    import ml_dtypes
    BF = ml_dtypes.bfloat16
    bl = hi - lo
    xt = np.ascontiguousarray(
        encoder_outputs[lo:hi].transpose(0, 2, 1)
    ).astype(BF)
    masklog = ((encoder_mask[lo:hi].astype(np.float32)) - 1.0) * 10000.0
    cov = np.ascontiguousarray(coverage[lo:hi], dtype=np.float32)
    dst = np.ascontiguousarray(
        decoder_state[lo:hi].T.reshape(4, 128, bl).transpose(1, 0, 2)
        .reshape(128, 4 * bl)
    ).astype(BF)
    d = {
        "xt": xt,
        "masklog": masklog.astype(BF),
        "covb": cov.astype(BF),
        "cov": cov,
        "dst": dst,
    }
    d.update(weights)
    return d


def run_on_cores(decoder_state, encoder_outputs, encoder_mask, coverage,
                 W_h, W_s, b_s, W_c, v, trace=False, tmpdir=None):
    from concourse.bass_utils import run_bass_kernel_spmd

    b_total = encoder_outputs.shape[0]
    s_len = encoder_outputs.shape[1]
    bl = b_total // N_CORES
    nc = _get_nc(bl, s_len)
    weights = _prep_weights(W_h, W_s, b_s, W_c, v)
    in_maps = [
        _prep_core_inputs(weights, decoder_state, encoder_outputs,
                          encoder_mask, coverage, i * bl, (i + 1) * bl)
        for i in range(N_CORES)
    ]
    res = run_bass_kernel_spmd(
        nc, in_maps, core_ids=list(range(N_CORES)), trace=trace, tmpdir=tmpdir
    )
    ctx = np.concatenate([res.results[i]["ctx"] for i in range(N_CORES)], axis=0)
    att = np.concatenate([res.results[i]["att"] for i in range(N_CORES)], axis=0)
    ncov = np.concatenate([res.results[i]["ncov"] for i in range(N_CORES)], axis=0)
    return (ctx, att, ncov), res


def kernel(decoder_state, encoder_outputs, encoder_mask, coverage,
           W_h, W_s, b_s, W_c, v):
    outs, _ = run_on_cores(
        np.asarray(decoder_state), np.asarray(encoder_outputs),
        np.asarray(encoder_mask), np.asarray(coverage),
        np.asarray(W_h), np.asarray(W_s), np.asarray(b_s),
        np.asarray(W_c), np.asarray(v),
    )
    return outs
